# revision 30
# baseline (speedup 1.0000x reference)
"""CAREConv Trainium2 kernel — 8-core SPMD Bass implementation (v2).

Pipeline (per core; nodes degree-sorted and snake-dealt to cores):
  A. t = tanh(feat @ W_mlp + b) for the core's node shard (PE matmul fp32 +
     ACT tanh), AllGather -> full t table (64-col padded rows) in DRAM.
  B. Per 4-tile group: bulk-gather t[src] for the ELL edge list via the
     gpsimd dma_gather (two int16 address windows, flex edges balanced
     between them host-side), L1 edge distance (DVE subtract + abs-sum
     reduce), composite sort key -(floor(dist*SCALE)*64 + slot) — exact in
     fp32, tie-free — iterated max8/match_replace gives the top-R*8 keys
     per node; slot recovered by mod 64; compacted gather indices into a
     host-staged edge-ordered feature table (featE[(col*128+lane)+1] =
     feat[src], row 0 = zeros) with count-mask folded in (c >= k -> row 0);
     per-tile feat gather of only R*8 candidate columns; contiguous
     tree-add -> scatter-mean; fused (feat + hr) @ W_lin + b on PE.
Host does integer/topology preprocessing only (ELL layout, degrees,
k = ceil(p*deg), window balancing, index lists, the edge-ordered feat
table permutation) and output unpermutation.
"""

import math
import os

import numpy as np

N_NODES = 50000
N_EDGES = 800000
D = 128
C = 40
CP = 64  # padded t row (256B)
P_KEEP = 0.5
NCORES = 8
TP = 128
NEG_BIG = -1.0e30
WIN = 32768  # int16 index window
TPG = 4
GCH = int(os.environ.get("K_GCH", "32"))  # t-gather chunk columns
PIPE = int(os.environ.get("K_PIPE", "1"))  # software-pipeline groups


def _wrap_idx(idx_ell):
    """[128, ncols] window-local indices -> wrapped [128, 8*ncols] int16.

    Gather-list position j = col*128 + p; wrapped[q, col*8 + r] =
    idx_ell[16*r + q, col]; replicated across the 8 16-partition groups.
    """
    ncols = idx_ell.shape[1]
    w = np.zeros((16, 8 * ncols), np.int16)
    for r in range(8):
        w[:, r::8] = idx_ell[16 * r : 16 * r + 16, :]
    return np.tile(w, (8, 1))


def _preprocess(src, dst, feat, n_nodes, ncores):
    E = src.shape[0]
    deg = np.bincount(dst, minlength=n_nodes).astype(np.int64)
    kk = (deg + 1) // 2

    perm = np.argsort(-deg, kind="stable")
    n_tiles = math.ceil(n_nodes / (TP * ncores)) * ncores
    NTOT = n_tiles * TP
    NPC = NTOT // ncores
    NSLOT = n_tiles // ncores

    t_idx = np.arange(n_tiles)
    rnd, pos = np.divmod(t_idx, ncores)
    core_of_tile = np.where(rnd % 2 == 0, pos, ncores - 1 - pos)

    fperm = np.full(NTOT, -1, np.int64)
    for c in range(ncores):
        tiles_c = t_idx[core_of_tile == c]
        for j, t in enumerate(tiles_c):
            ids = perm[t * TP : min((t + 1) * TP, n_nodes)]
            base = c * NPC + j * TP
            fperm[base : base + len(ids)] = ids
    valid = fperm >= 0
    inv_f = np.full(n_nodes, -1, np.int64)
    inv_f[fperm[valid]] = np.nonzero(valid)[0]

    degf = np.zeros(NTOT, np.int64)
    degf[valid] = deg[fperm[valid]]
    kf = np.zeros(NTOT, np.int64)
    kf[valid] = kk[fperm[valid]]

    dstf = inv_f[dst]
    srcv = inv_f[src]

    # window A = t rows [0, WIN); window B = t rows [NTOT-WIN, NTOT).
    # overlap [NTOT-WIN, WIN) edges are assigned to balance the two sides
    # per dst node (minimizes ELL width SA+SB).
    onlyB = srcv >= WIN
    flexm = (srcv >= NTOT - WIN) & ~onlyB
    onlyA = ~onlyB & ~flexm
    aA = np.bincount(dstf[onlyA], minlength=NTOT)
    aB = np.bincount(dstf[onlyB], minlength=NTOT)
    fl = np.bincount(dstf[flexm], minlength=NTOT)
    x = np.clip((aB + fl - aA + 1) // 2, 0, fl)  # flex edges sent to side A

    side = onlyB.astype(np.int64)  # 0 = A, 1 = B
    fidx = np.nonzero(flexm)[0]
    fo = np.lexsort((fidx, dstf[fidx]))
    fsorted = fidx[fo]
    grp = dstf[fsorted]
    gstart = np.zeros(NTOT + 1, np.int64)
    np.cumsum(np.bincount(grp, minlength=NTOT), out=gstart[1:])
    frank = np.arange(len(fsorted)) - gstart[grp]
    side[fsorted] = (frank >= x[grp]).astype(np.int64)

    cA = np.bincount(dstf[side == 0], minlength=NTOT)
    cB = degf - cA

    cA3 = cA.reshape(ncores, NSLOT, TP)
    cB3 = cB.reshape(ncores, NSLOT, TP)
    kf3 = kf.reshape(ncores, NSLOT, TP)
    SA_list = [int(cA3[:, j, :].max()) for j in range(NSLOT)]
    SB_list = [int(cB3[:, j, :].max()) for j in range(NSLOT)]
    S_list = [max(a + b, 8) for a, b in zip(SA_list, SB_list)]
    SA_list = [s - b for s, b in zip(S_list, SB_list)]  # keep S = SA + SB
    R_list = [max(1, (int(kf3[:, j, :].max()) + 7) // 8) for j in range(NSLOT)]
    offs = np.concatenate([[0], np.cumsum(S_list)]).astype(np.int64)
    F_tot = int(offs[-1])
    SW = max(S_list)
    CW = max(R_list) * 8

    # slot of each edge: A edges at [0, cA_v), B edges at [SA_j, SA_j + cB_v)
    order = np.lexsort((np.arange(E), side, dstf))
    dst_s = dstf[order]
    src_s = srcv[order]
    side_s = side[order]
    src_orig_s = src[order]
    segc = np.bincount(dst_s, minlength=NTOT)
    offs_seg = np.zeros(NTOT + 1, np.int64)
    np.cumsum(segc, out=offs_seg[1:])
    pos_in_seg = np.arange(E) - offs_seg[dst_s]
    sa_of_node = np.repeat(np.tile(np.array(SA_list), ncores), TP)
    slot = np.where(side_s == 0, pos_in_seg,
                    sa_of_node[dst_s] + pos_in_seg - cA[dst_s])

    ell_t = np.zeros((NTOT, SW), np.int32)     # window-local t index
    ell_fsrc = np.full((NTOT, SW), -1, np.int64)  # original src node id
    real = np.zeros((NTOT, SW), bool)
    tidx_loc = np.where(side_s == 0, src_s, src_s - (NTOT - WIN)).astype(np.int32)
    ell_t[dst_s, slot] = tidx_loc
    ell_fsrc[dst_s, slot] = src_orig_s
    real[dst_s, slot] = True

    groups = [(g, min(g + TPG, NSLOT)) for g in range(0, NSLOT, TPG)]
    feat = np.ascontiguousarray(feat, dtype=np.float32)

    # per-group gather-call layout for the t gather (A chunk | B chunk,
    # rounded to GCH columns, trailing pad columns idx=-1 + num_idxs_reg)
    gmeta = []
    for (g0, g1) in groups:
        nA = sum(SA_list[g0:g1])
        nB = sum(SB_list[g0:g1])
        padA = math.ceil(nA / GCH) * GCH if nA else 0
        padB = math.ceil(nB / GCH) * GCH if nB else 0
        gmeta.append({"nA": nA, "nB": nB, "padA": padA, "padB": padB})
    wrap_off = np.concatenate(
        [[0], np.cumsum([m["padA"] + m["padB"] for m in gmeta])]).astype(int)
    febase = np.concatenate(
        [[0], np.cumsum([1 + (offs[g1] - offs[g0]) * TP
                         for (g0, g1) in groups])]).astype(int)
    FE_rows = int(febase[-1])

    in_maps = []
    for c in range(ncores):
        vids = np.arange(c * NPC, (c + 1) * NPC)
        wparts = []
        featE = np.zeros((FE_rows, D), np.float16)
        for gi, (g0, g1) in enumerate(groups):
            m = gmeta[gi]
            colsA = np.full((TP, m["padA"]), -1, np.int32)
            colsB = np.full((TP, m["padB"]), -1, np.int32)
            ca = cb = 0
            for j in range(g0, g1):
                vj = vids[j * TP : (j + 1) * TP]
                sa, sb = SA_list[j], SB_list[j]
                et = ell_t[vj]
                rl = real[vj]
                colsA[:, ca : ca + sa] = np.where(rl[:, :sa], et[:, :sa], 0)
                colsB[:, cb : cb + sb] = np.where(
                    rl[:, sa : sa + sb], et[:, sa : sa + sb], 0)
                # featE rows for this tile's columns
                a0 = int(offs[j] - offs[g0])
                fs = ell_fsrc[vj, : sa + sb]  # [TP, S_j]
                rows = (febase[gi] + 1 + (a0 + np.arange(sa + sb))[None, :] * TP
                        + np.arange(TP)[:, None])
                rmask = fs >= 0
                featE[rows[rmask]] = feat[fs[rmask]]
                ca += sa
                cb += sb
            wparts.append(_wrap_idx(np.concatenate([colsA, colsB], axis=1)))
        srctW = np.concatenate(wparts, axis=1) if wparts else np.zeros(
            (TP, 8), np.int16)

        # bigp in per-tile-contiguous (nd) layout
        bigp = np.full((TP, F_tot), NEG_BIG, np.float32)
        for j in range(NSLOT):
            vj = vids[j * TP : (j + 1) * TP]
            S_j = S_list[j]
            bigp[:, offs[j] : offs[j] + S_j] = np.where(
                real[vj, :S_j], 0.0, NEG_BIG)

        kfc = kf[vids].reshape(NSLOT, TP).T.astype(np.float32)
        degc = degf[vids].reshape(NSLOT, TP).T
        am = np.where(degc > 0, 1.0 / np.maximum(kfc, 1.0), 0.0).astype(np.float32)
        c1 = np.where(degc > 0, 1.0, 2.0).astype(np.float32)
        cmaskW = np.concatenate(
            [(np.arange(8 * R_list[j])[None, :] < kfc[:, j][:, None])
             for j in range(NSLOT)], axis=1).astype(np.float32)
        feat_pad = np.zeros((NPC, D), np.float32)
        vmask = valid[vids]
        feat_pad[vmask] = feat[fperm[vids[vmask]]]
        featT = feat_pad.T.copy()
        featN = (feat_pad.reshape(NSLOT, TP, D).transpose(1, 0, 2)
                 .reshape(TP, -1)).copy()
        m8 = (np.arange(TP)[:, None] // 16 == np.arange(8)[None, :]).astype(
            np.float32)
        m16 = (np.arange(TP)[:, None] % 16 == np.arange(TP)[None, :] % 16).astype(
            np.float32)
        in_maps.append(
            {
                "m8": m8,
                "m16": m16,
                "featT": featT,
                "featN": featN,
                "featE": featE,
                "srctW": srctW,
                "bigp": bigp,
                "kf": kfc,
                "am": am,
                "c1": c1,
                "cmaskW": cmaskW,
            }
        )

    sched = {
        "NTOT": NTOT,
        "NPC": NPC,
        "NSLOT": NSLOT,
        "SA": SA_list,
        "SB": SB_list,
        "S": S_list,
        "R": R_list,
        "offs": offs.tolist(),
        "F_tot": F_tot,
        "groups": groups,
        "gmeta": gmeta,
        "wrap_off": wrap_off.tolist(),
        "febase": febase.tolist(),
        "FE_rows": FE_rows,
        "SW": SW,
        "CW": CW,
        "c8off": np.concatenate(
            [[0], np.cumsum([8 * r for r in R_list])]).astype(int).tolist(),
    }
    return sched, in_maps, fperm, valid


# ----------------------------------------------------------------------------
# Bass program builder (SPMD: one program; per-core variation is data only)
# ----------------------------------------------------------------------------
def _build_bass(sched, ncores):
    STAGE = int(os.environ.get("K_STAGE", "99"))
    import concourse.bass as bass
    import concourse.bacc as bacc
    import concourse.tile as tile
    from concourse import mybir
    from concourse.masks import make_identity

    f32 = mybir.dt.float32
    f16 = mybir.dt.float16
    i16 = mybir.dt.int16
    CPH = 2 * CP  # fp16 t row: 128 halves = 256B
    NTOT, NPC, NSLOT = sched["NTOT"], sched["NPC"], sched["NSLOT"]
    SA_list, SB_list, S_list, R_list = (
        sched["SA"], sched["SB"], sched["S"], sched["R"],
    )
    offs, F_tot, groups = sched["offs"], sched["F_tot"], sched["groups"]
    gmeta, wrap_off, febase = sched["gmeta"], sched["wrap_off"], sched["febase"]
    FE_rows, SW, CW = sched["FE_rows"], sched["SW"], sched["CW"]
    c8off = sched["c8off"]
    R8_tot = c8off[-1]

    nc = bacc.Bacc(None, num_devices=ncores)

    featT = nc.dram_tensor("featT", [D, NPC], f32, kind="ExternalInput")
    featN = nc.dram_tensor("featN", [TP, NSLOT * D], f32, kind="ExternalInput")
    featE = nc.dram_tensor("featE", [FE_rows, D], f16, kind="ExternalInput")
    srctW = nc.dram_tensor("srctW", [TP, 8 * wrap_off[-1]], i16,
                           kind="ExternalInput")
    bigp = nc.dram_tensor("bigp", [TP, F_tot], f32, kind="ExternalInput")
    kf_d = nc.dram_tensor("kf", [TP, NSLOT], f32, kind="ExternalInput")
    cmask_d = nc.dram_tensor("cmaskW", [TP, R8_tot], f32, kind="ExternalInput")
    am_d = nc.dram_tensor("am", [TP, NSLOT], f32, kind="ExternalInput")
    c1_d = nc.dram_tensor("c1", [TP, NSLOT], f32, kind="ExternalInput")
    wmlp = nc.dram_tensor("wmlp", [D, C], f32, kind="ExternalInput")
    bmlp = nc.dram_tensor("bmlp", [C, 1], f32, kind="ExternalInput")
    wlin = nc.dram_tensor("wlin", [D, D], f32, kind="ExternalInput")
    blin = nc.dram_tensor("blin", [D, 1], f32, kind="ExternalInput")
    m8_d = nc.dram_tensor("m8", [TP, 8], f32, kind="ExternalInput")
    m16_d = nc.dram_tensor("m16", [TP, TP], f32, kind="ExternalInput")
    outT = nc.dram_tensor("outT", [D, NPC], f32, kind="ExternalOutput")

    with tile.TileContext(nc) as tc:
        with (
            tc.tile_pool(name="persist", bufs=1) as pp,
            tc.tile_pool(name="dram", bufs=1, space="DRAM") as dp,
        ):
            t_loc = dp.tile([NPC, CPH], f16)
            t_aug = dp.tile([NTOT, CPH], f16)

            ident = pp.tile([TP, TP], f32)
            make_identity(nc, ident[:])
            wmlp_t = pp.tile([D, C], f32)
            nc.sync.dma_start(wmlp_t[:], wmlp[:])
            bmlp_t = pp.tile([C, 1], f32)
            nc.sync.dma_start(bmlp_t[:], bmlp[:])
            wlin_t = pp.tile([D, D], f32)
            nc.sync.dma_start(wlin_t[:], wlin[:])
            blin_t = pp.tile([D, 1], f32)
            nc.sync.dma_start(blin_t[:], blin[:])
            kf_t = pp.tile([TP, NSLOT], f32)
            nc.sync.dma_start(kf_t[:], kf_d[:])
            cmask_t = pp.tile([TP, R8_tot], f32)
            nc.sync.dma_start(cmask_t[:], cmask_d[:])
            am_t = pp.tile([TP, NSLOT], f32)
            nc.sync.dma_start(am_t[:], am_d[:])
            c1_t = pp.tile([TP, NSLOT], f32)
            nc.sync.dma_start(c1_t[:], c1_d[:])
            m8_t = pp.tile([TP, 8], f32)
            nc.sync.dma_start(m8_t[:], m8_d[:])
            m16_t = pp.tile([TP, TP], f32)
            nc.sync.dma_start(m16_t[:], m16_d[:])

            iota_i = pp.tile([TP, SW], mybir.dt.int32)
            nc.gpsimd.iota(out=iota_i[:], pattern=[[1, SW]], base=0,
                           channel_multiplier=0)
            iota_f = pp.tile([TP, SW], f32)
            nc.vector.tensor_copy(out=iota_f[:], in_=iota_i[:])
            iotap_i = pp.tile([TP, 1], mybir.dt.int32)
            nc.gpsimd.iota(out=iotap_i[:], pattern=[[0, 1]], base=0,
                           channel_multiplier=1)
            iota_p = pp.tile([TP, 1], f32)
            nc.vector.tensor_copy(out=iota_p[:], in_=iotap_i[:])
            t_own = pp.tile([TP, NSLOT * C], f16)

            # ---------------- Phase A: t = tanh(feat @ W_mlp + b) ----------
            with (
                tc.tile_pool(name="pa", bufs=3) as pa,
                tc.tile_pool(name="psA", bufs=2, space="PSUM") as psA,
            ):
                o = 0
                while o < NPC:
                    w = min(512, NPC - o)
                    ft = pa.tile([D, w], f32, tag="ft")
                    nc.sync.dma_start(ft[:], featT[:, o : o + w])
                    pm = psA.tile([C, w], f32, tag="pm")
                    nc.tensor.matmul(out=pm[:], lhsT=wmlp_t[:], rhs=ft[:],
                                     start=True, stop=True)
                    th = pa.tile([C, w], f32, tag="th")
                    nc.scalar.activation(out=th[:], in_=pm[:],
                                         func=mybir.ActivationFunctionType.Tanh,
                                         bias=bmlp_t[:])
                    for i in range(w // TP):
                        j = (o + i * TP) // TP
                        pt = psA.tile([TP, C], f32, tag="pt")
                        nc.tensor.transpose(out=pt[:], in_=th[:, i * TP : (i + 1) * TP],
                                            identity=ident[:C, :C])
                        nc.vector.tensor_copy(out=t_own[:, j * C : (j + 1) * C],
                                              in_=pt[:])
                        nc.sync.dma_start(
                            t_loc[o + i * TP : o + (i + 1) * TP, :C],
                            t_own[:, j * C : (j + 1) * C],
                        )
                    o += w

            nc.gpsimd.collective_compute(
                "AllGather",
                mybir.AluOpType.bypass,
                replica_groups=[list(range(ncores))],
                ins=[t_loc[:].opt()],
                outs=[t_aug[:].opt()],
            )

            if STAGE < 4:
                for j in range(NSLOT):
                    nc.sync.dma_start(outT[:, j * TP : (j + 1) * TP], ident[:])
            # ---------------- Phase B ---------------------------------------
            # Software-pipelined: loads + t-gathers of group g+1 are emitted
            # before the compute of group g, so the in-order Pool queue can
            # generate the next group's gather descriptors while the DVE
            # works on the current group (the feat gather of group g is the
            # only Pool instruction that waits on DVE results).
            with (
                tc.tile_pool(name="pb", bufs=2) as pb,
                tc.tile_pool(name="pfin", bufs=3) as pfin,
                tc.tile_pool(name="pmsg", bufs=2) as pmsg,
                tc.tile_pool(name="psB", bufs=2, space="PSUM") as psB,
            ):

                def emit_load(gi):
                    j0, j1 = groups[gi]
                    nt = j1 - j0
                    Wg = offs[j1] - offs[j0]
                    gm = gmeta[gi]
                    nA, nB = gm["nA"], gm["nB"]
                    padA, padB = gm["padA"], gm["padB"]

                    big = pb.tile([TP, Wg], f32, tag="big")
                    nc.sync.dma_start(big[:], bigp[:, offs[j0] : offs[j1]])
                    fno = pfin.tile([TP, nt * D], f32, tag="fno")
                    nc.sync.dma_start(fno[:], featN[:, j0 * D : j1 * D])
                    wiT = pb.tile([TP, 8 * (padA + padB)], i16, tag="wiT")
                    nc.sync.dma_start(
                        wiT[:],
                        srctW[:, 8 * wrap_off[gi] : 8 * wrap_off[gi + 1]])

                    tsrc = pb.tile([TP, (padA + padB) * CPH], f16, tag="tsrc")
                    tsrc3 = tsrc[:].rearrange("p (w c) -> p w c", c=CPH)
                    for c0 in range(0, padA, GCH):
                        rcols = min(GCH, nA - c0)
                        nc.gpsimd.dma_gather(
                            out_ap=tsrc3[:, c0 : c0 + GCH, :],
                            in_ap=t_aug[:WIN, :],
                            idxs_ap=wiT[:, 8 * c0 : 8 * (c0 + GCH)],
                            num_idxs=GCH * TP, num_idxs_reg=rcols * TP,
                            elem_size=CPH,
                            single_packet=(GCH * TP <= 1024),
                        )
                    for c0 in range(0, padB, GCH):
                        rcols = min(GCH, nB - c0)
                        nc.gpsimd.dma_gather(
                            out_ap=tsrc3[:, padA + c0 : padA + c0 + GCH, :],
                            in_ap=t_aug[NTOT - WIN : NTOT, :],
                            idxs_ap=wiT[:, 8 * (padA + c0) : 8 * (padA + c0 + GCH)],
                            num_idxs=GCH * TP, num_idxs_reg=rcols * TP,
                            elem_size=CPH,
                            single_packet=(GCH * TP <= 1024),
                        )
                    return {"big": big, "fno": fno, "tsrc": tsrc,
                            "tsrc3": tsrc3}

                def emit_sel(gi, tl):
                    j0, j1 = groups[gi]
                    nt = j1 - j0
                    Wg = offs[j1] - offs[j0]
                    gm = gmeta[gi]
                    padA, padB = gm["padA"], gm["padB"]
                    R8g = sum(8 * R_list[j] for j in range(j0, j1))
                    big = tl["big"]
                    tsrc, tsrc3 = tl["tsrc"], tl["tsrc3"]

                    # subtract own-node t (broadcast over slots) per tile/part
                    for tci in range(nt):
                        j = j0 + tci
                        sa, sb = SA_list[j], SB_list[j]
                        tdst_b = t_own[:, j * C : (j + 1) * C]
                        cA0 = sum(SA_list[j0:j])
                        cB0 = padA + sum(SB_list[j0:j])
                        for (c0, ns) in ((cA0, sa), (cB0, sb)):
                            if ns == 0:
                                continue
                            sl = tsrc[:, c0 * CPH : (c0 + ns) * CPH].rearrange(
                                "p (s c) -> p s c", s=ns)[:, :, :C]
                            nc.vector.tensor_tensor(
                                out=sl, in0=sl,
                                in1=tdst_b.unsqueeze(1).to_broadcast([TP, ns, C]),
                                op=mybir.AluOpType.subtract,
                            )
                    ndg = pb.tile([TP, padA + padB], f32, tag="ndg")
                    nA_g, nB_g = gm["nA"], gm["nB"]
                    nc.vector.tensor_reduce(
                        out=ndg[:, :nA_g],
                        in_=tsrc3[:, :nA_g, :C],
                        axis=mybir.AxisListType.X, op=mybir.AluOpType.add,
                        apply_absolute_value=True, negate=True,
                    )
                    nc.vector.tensor_reduce(
                        out=ndg[:, padA : padA + nB_g],
                        in_=tsrc3[:, padA : padA + nB_g, :C],
                        axis=mybir.AxisListType.X, op=mybir.AluOpType.add,
                        apply_absolute_value=True, negate=True,
                    )
                    # per-tile contiguous negated dists (A|B adjacent) + pad
                    # mask (-1e30); top-R*8 per node via max8/match_replace;
                    # max_index recovers the slot positions (ties only for
                    # duplicate edges, where either index gives the same row).
                    ndw = pb.tile([TP, Wg], f32, tag="ndw")
                    for tci in range(nt):
                        j = j0 + tci
                        sa, sb = SA_list[j], SB_list[j]
                        a0 = offs[j] - offs[j0]
                        cA0 = sum(SA_list[j0:j])
                        cB0 = padA + sum(SB_list[j0:j])
                        if sa:
                            nc.vector.tensor_copy(out=ndw[:, a0 : a0 + sa],
                                                  in_=ndg[:, cA0 : cA0 + sa])
                        if sb:
                            nc.vector.tensor_copy(out=ndw[:, a0 + sa : a0 + sa + sb],
                                                  in_=ndg[:, cB0 : cB0 + sb])
                    nc.vector.tensor_tensor(out=ndw[:], in0=ndw[:], in1=big[:],
                                            op=mybir.AluOpType.add)
                    cand = pb.tile([TP, CW], f32, tag="cand")
                    idxu = pb.tile([TP, CW], mybir.dt.uint16, tag="idxu")
                    gsel = pb.tile([TP, R8g], f32, tag="gsel")
                    r8off = 0
                    for tci in range(nt):
                        j = j0 + tci
                        S_j = S_list[j]
                        R8j = 8 * R_list[j]
                        a0 = offs[j] - offs[j0]
                        ndw_j = ndw[:, a0 : a0 + S_j]
                        for r in range(R_list[j]):
                            nc.vector.max(out=cand[:, r * 8 : (r + 1) * 8],
                                          in_=ndw_j)
                            nc.vector.max_index(
                                out=idxu[:, r * 8 : (r + 1) * 8],
                                in_max=cand[:, r * 8 : (r + 1) * 8],
                                in_values=ndw_j)
                            if r < R_list[j] - 1:
                                nc.vector.match_replace(
                                    out=ndw_j,
                                    in_to_replace=cand[:, r * 8 : (r + 1) * 8],
                                    in_values=ndw_j, imm_value=NEG_BIG)
                        gsl = gsel[:, r8off : r8off + R8j]
                        # gidx = (a0 + slot)*128 + p + 1, zeroed for c >= k
                        nc.vector.tensor_copy(out=gsl, in_=idxu[:, :R8j])
                        nc.vector.tensor_scalar(
                            out=gsl, in0=gsl, scalar1=float(TP),
                            scalar2=float(a0 * TP + 1), op0=mybir.AluOpType.mult,
                            op1=mybir.AluOpType.add)
                        nc.vector.tensor_tensor(
                            out=gsl, in0=gsl,
                            in1=iota_p[:].to_broadcast([TP, R8j]),
                            op=mybir.AluOpType.add)
                        nc.vector.tensor_tensor(
                            out=gsl, in0=gsl,
                            in1=cmask_t[:, c8off[j] : c8off[j] + R8j],
                            op=mybir.AluOpType.mult)
                        r8off += R8j

                    # wrapped int16 transform of gsel for the whole group
                    bsel = pb.tile([TP, 8 * R8g], f32, tag="bsel")
                    nc.vector.tensor_tensor(
                        out=bsel[:].rearrange("p (s r) -> p s r", r=8),
                        in0=gsel[:].unsqueeze(2).to_broadcast([TP, R8g, 8]),
                        in1=m8_t[:].unsqueeze(1).to_broadcast([TP, R8g, 8]),
                        op=mybir.AluOpType.mult)
                    wmi = pb.tile([TP, 8 * R8g], i16, tag="wmi")
                    for ch in range(0, 8 * R8g, 512):
                        cw = min(512, 8 * R8g - ch)
                        pw = psB.tile([TP, 512], f32, tag="pwrap")
                        nc.tensor.matmul(out=pw[:, :cw], lhsT=m16_t[:],
                                         rhs=bsel[:, ch : ch + cw],
                                         start=True, stop=True)
                        nc.vector.tensor_copy(out=wmi[:, ch : ch + cw],
                                              in_=pw[:, :cw])
                    return {"wmi": wmi}

                def emit_fin(gi, tl, sl):
                    j0, j1 = groups[gi]
                    nt = j1 - j0
                    R8g = sum(8 * R_list[j] for j in range(j0, j1))
                    fno = tl["fno"]
                    wmi = sl["wmi"]

                    # feat gather per tile pair; per tile tree-add + linear
                    feg = featE[febase[gi] : febase[gi + 1], :]
                    r8off = 0
                    for tp0 in range(0, nt, 2):
                        tps = [t for t in (tp0, tp0 + 1) if t < nt]
                        R8p = sum(8 * R_list[j0 + t] for t in tps)
                        msgp = pmsg.tile([TP, R8p * D], f16, tag="msg")
                        nc.gpsimd.dma_gather(
                            out_ap=msgp[:].rearrange("p (s d) -> p s d", d=D),
                            in_ap=feg,
                            idxs_ap=wmi[:, 8 * r8off : 8 * (r8off + R8p)],
                            num_idxs=R8p * TP, num_idxs_reg=R8p * TP,
                            elem_size=D,
                            single_packet=(R8p * TP <= 1024),
                        )
                        r8off += R8p
                        moff = 0
                        for tci in tps:
                            emit_tile_fin(gi, tci, msgp, moff, fno)
                            moff += 8 * R_list[j0 + tci] * D

                def emit_tile_fin(gi, tci, msgp, moff, fno):
                    j0, j1 = groups[gi]
                    j = j0 + tci
                    R8j = 8 * R_list[j]
                    msg = msgp[:, moff : moff + R8j * D]
                    # contiguous tree-add over the R8j candidate columns
                    if True:
                        w = R8j
                        while w > 1:
                            h = w // 2
                            nc.vector.tensor_tensor(
                                out=msg[:, : h * D], in0=msg[:, : h * D],
                                in1=msg[:, h * D : 2 * h * D],
                                op=mybir.AluOpType.add)
                            if w % 2:
                                nc.vector.tensor_tensor(
                                    out=msg[:, :D], in0=msg[:, :D],
                                    in1=msg[:, (w - 1) * D : w * D],
                                    op=mybir.AluOpType.add)
                            w = h
                        h_t = pmsg.tile([TP, D], f32, tag="h")
                        nc.vector.tensor_scalar(
                            out=h_t[:], in0=msg[:, :D], scalar1=am_t[:, j : j + 1],
                            scalar2=None, op0=mybir.AluOpType.mult)
                        fown = pmsg.tile([TP, D], f32, tag="fown")
                        nc.vector.tensor_scalar(
                            out=fown[:], in0=fno[:, tci * D : (tci + 1) * D],
                            scalar1=c1_t[:, j : j + 1],
                            scalar2=None, op0=mybir.AluOpType.mult)
                        nc.vector.tensor_tensor(out=h_t[:], in0=h_t[:], in1=fown[:],
                                                op=mybir.AluOpType.add)
                        hT_p = psB.tile([TP, D], f32, tag="hT_p")
                        nc.tensor.transpose(out=hT_p[:], in_=h_t[:],
                                            identity=ident[:])
                        hT = pmsg.tile([TP, D], f32, tag="hT")
                        nc.vector.tensor_copy(out=hT[:], in_=hT_p[:])
                        op = psB.tile([D, TP], f32, tag="op")
                        nc.tensor.matmul(out=op[:], lhsT=wlin_t[:], rhs=hT[:],
                                         start=True, stop=True)
                        ot = pmsg.tile([D, TP], f32, tag="ot")
                        nc.scalar.activation(
                            out=ot[:], in_=op[:],
                            func=mybir.ActivationFunctionType.Identity,
                            bias=blin_t[:])
                        nc.sync.dma_start(outT[:, j * TP : (j + 1) * TP], ot[:])

                if STAGE > 1:
                    ng = len(groups)
                    if PIPE:
                        tls, sls = {}, {}
                        for gi in range(ng + 2):
                            if gi < ng:
                                tls[gi] = emit_load(gi)
                            if 1 <= gi <= ng and STAGE > 2:
                                sls[gi - 1] = emit_sel(gi - 1, tls[gi - 1])
                            if gi >= 2 and STAGE > 3:
                                emit_fin(gi - 2, tls[gi - 2], sls[gi - 2])
                    else:
                        for gi in range(ng):
                            tl = emit_load(gi)
                            if STAGE > 2:
                                sl = emit_sel(gi, tl)
                                if STAGE > 3:
                                    emit_fin(gi, tl, sl)
    nc.finalize()
    return nc


# ----------------------------------------------------------------------------
# Runner
# ----------------------------------------------------------------------------
_CACHE = {}


def _get_program(sched, ncores):
    key = (tuple(sched["S"]), tuple(sched["SA"]), tuple(sched["R"]),
           sched["NTOT"], GCH, PIPE)
    if key not in _CACHE:
        _CACHE[key] = _build_bass(sched, ncores)
    return _CACHE[key]


def kernel(**inputs):
    feat = np.asarray(inputs["feat"], dtype=np.float32)
    src = np.asarray(inputs["src"]).astype(np.int64)
    dst = np.asarray(inputs["dst"]).astype(np.int64)
    W_mlp = np.asarray(inputs["W_mlp"], dtype=np.float32)
    b_mlp = np.asarray(inputs["b_mlp"], dtype=np.float32)
    W_lin = np.asarray(inputs["W_lin"], dtype=np.float32)
    b_lin = np.asarray(inputs["b_lin"], dtype=np.float32)
    n = feat.shape[0]

    sched, in_maps, fperm, valid = _preprocess(src, dst, feat, n, NCORES)
    for m in in_maps:
        m["wmlp"] = np.ascontiguousarray(W_mlp)
        m["bmlp"] = np.ascontiguousarray(b_mlp.reshape(C, 1))
        m["wlin"] = np.ascontiguousarray(W_lin)
        m["blin"] = np.ascontiguousarray(b_lin.reshape(D, 1))

    nc = _get_program(sched, NCORES)

    from concourse.bass_utils import run_bass_kernel_spmd

    res = run_bass_kernel_spmd(nc, in_maps, list(range(NCORES)))

    full = np.concatenate([res.results[c]["outT"] for c in range(NCORES)], axis=1)
    out = np.empty((n, D), np.float32)
    out[fperm[valid]] = full.T[valid]
    return out


# revision 33
# speedup vs baseline: 1.5576x; 1.5576x over previous
"""CAREConv Trainium2 kernel — 8-core SPMD Bass implementation (v2).

Pipeline (per core; nodes degree-sorted and snake-dealt to cores):
  A. t = tanh(feat @ W_mlp + b) for the core's node shard (PE matmul fp32 +
     ACT tanh), AllGather -> full t table (64-col padded rows) in DRAM.
  B. Per 4-tile group: bulk-gather t[src] for the ELL edge list via the
     gpsimd dma_gather (two int16 address windows, flex edges balanced
     between them host-side), L1 edge distance (DVE subtract + abs-sum
     reduce), composite sort key -(floor(dist*SCALE)*64 + slot) — exact in
     fp32, tie-free — iterated max8/match_replace gives the top-R*8 keys
     per node; slot recovered by mod 64; compacted gather indices into a
     host-staged edge-ordered feature table (featE[(col*128+lane)+1] =
     feat[src], row 0 = zeros) with count-mask folded in (c >= k -> row 0);
     per-tile feat gather of only R*8 candidate columns; contiguous
     tree-add -> scatter-mean; fused (feat + hr) @ W_lin + b on PE.
Host does integer/topology preprocessing only (ELL layout, degrees,
k = ceil(p*deg), window balancing, index lists, the edge-ordered feat
table permutation) and output unpermutation.
"""

import math
import os

import numpy as np

N_NODES = 50000
N_EDGES = 800000
D = 128
C = 40
CP = 64  # padded t row (256B)
P_KEEP = 0.5
NCORES = 8
TP = 128
NEG_BIG = -1.0e30
WIN = 32768  # int16 index window
TPG = 4
GCH = int(os.environ.get("K_GCH", "32"))  # t-gather chunk columns
PIPE = int(os.environ.get("K_PIPE", "1"))  # software-pipeline groups


def _wrap_idx(idx_ell):
    """[128, ncols] window-local indices -> wrapped [128, 8*ncols] int16.

    Gather-list position j = col*128 + p; wrapped[q, col*8 + r] =
    idx_ell[16*r + q, col]; replicated across the 8 16-partition groups.
    """
    ncols = idx_ell.shape[1]
    w = np.zeros((16, 8 * ncols), np.int16)
    for r in range(8):
        w[:, r::8] = idx_ell[16 * r : 16 * r + 16, :]
    return np.tile(w, (8, 1))


def _preprocess(src, dst, feat, n_nodes, ncores):
    E = src.shape[0]
    deg = np.bincount(dst, minlength=n_nodes).astype(np.int64)
    kk = (deg + 1) // 2

    perm = np.argsort(-deg, kind="stable")
    n_tiles = math.ceil(n_nodes / (TP * ncores)) * ncores
    NTOT = n_tiles * TP
    NPC = NTOT // ncores
    NSLOT = n_tiles // ncores

    t_idx = np.arange(n_tiles)
    rnd, pos = np.divmod(t_idx, ncores)
    core_of_tile = np.where(rnd % 2 == 0, pos, ncores - 1 - pos)

    fperm = np.full(NTOT, -1, np.int64)
    for c in range(ncores):
        tiles_c = t_idx[core_of_tile == c]
        for j, t in enumerate(tiles_c):
            ids = perm[t * TP : min((t + 1) * TP, n_nodes)]
            base = c * NPC + j * TP
            fperm[base : base + len(ids)] = ids
    valid = fperm >= 0
    inv_f = np.full(n_nodes, -1, np.int64)
    inv_f[fperm[valid]] = np.nonzero(valid)[0]

    degf = np.zeros(NTOT, np.int64)
    degf[valid] = deg[fperm[valid]]
    kf = np.zeros(NTOT, np.int64)
    kf[valid] = kk[fperm[valid]]

    dstf = inv_f[dst]
    srcv = inv_f[src]

    # window A = t rows [0, WIN); window B = t rows [NTOT-WIN, NTOT).
    # overlap [NTOT-WIN, WIN) edges are assigned to balance the two sides
    # per dst node (minimizes ELL width SA+SB).
    onlyB = srcv >= WIN
    flexm = (srcv >= NTOT - WIN) & ~onlyB
    onlyA = ~onlyB & ~flexm
    aA = np.bincount(dstf[onlyA], minlength=NTOT)
    aB = np.bincount(dstf[onlyB], minlength=NTOT)
    fl = np.bincount(dstf[flexm], minlength=NTOT)
    x = np.clip((aB + fl - aA + 1) // 2, 0, fl)  # flex edges sent to side A

    side = onlyB.astype(np.int64)  # 0 = A, 1 = B
    fidx = np.nonzero(flexm)[0]
    fo = np.lexsort((fidx, dstf[fidx]))
    fsorted = fidx[fo]
    grp = dstf[fsorted]
    gstart = np.zeros(NTOT + 1, np.int64)
    np.cumsum(np.bincount(grp, minlength=NTOT), out=gstart[1:])
    frank = np.arange(len(fsorted)) - gstart[grp]
    side[fsorted] = (frank >= x[grp]).astype(np.int64)

    cA = np.bincount(dstf[side == 0], minlength=NTOT)
    cB = degf - cA

    cA3 = cA.reshape(ncores, NSLOT, TP)
    cB3 = cB.reshape(ncores, NSLOT, TP)
    kf3 = kf.reshape(ncores, NSLOT, TP)
    SA_list = [int(cA3[:, j, :].max()) for j in range(NSLOT)]
    SB_list = [int(cB3[:, j, :].max()) for j in range(NSLOT)]
    S_list = [max(a + b, 8) for a, b in zip(SA_list, SB_list)]
    SA_list = [s - b for s, b in zip(S_list, SB_list)]  # keep S = SA + SB
    R_list = [max(1, (int(kf3[:, j, :].max()) + 7) // 8) for j in range(NSLOT)]
    offs = np.concatenate([[0], np.cumsum(S_list)]).astype(np.int64)
    F_tot = int(offs[-1])
    SW = max(S_list)
    CW = max(R_list) * 8

    # slot of each edge: A edges at [0, cA_v), B edges at [SA_j, SA_j + cB_v)
    order = np.lexsort((np.arange(E), side, dstf))
    dst_s = dstf[order]
    src_s = srcv[order]
    side_s = side[order]
    src_orig_s = src[order]
    segc = np.bincount(dst_s, minlength=NTOT)
    offs_seg = np.zeros(NTOT + 1, np.int64)
    np.cumsum(segc, out=offs_seg[1:])
    pos_in_seg = np.arange(E) - offs_seg[dst_s]
    sa_of_node = np.repeat(np.tile(np.array(SA_list), ncores), TP)
    slot = np.where(side_s == 0, pos_in_seg,
                    sa_of_node[dst_s] + pos_in_seg - cA[dst_s])

    ell_t = np.zeros((NTOT, SW), np.int32)     # window-local t index
    ell_fsrc = np.full((NTOT, SW), -1, np.int64)  # original src node id
    real = np.zeros((NTOT, SW), bool)
    tidx_loc = np.where(side_s == 0, src_s, src_s - (NTOT - WIN)).astype(np.int32)
    ell_t[dst_s, slot] = tidx_loc
    ell_fsrc[dst_s, slot] = src_orig_s
    real[dst_s, slot] = True

    groups = [(g, min(g + TPG, NSLOT)) for g in range(0, NSLOT, TPG)]
    feat = np.ascontiguousarray(feat, dtype=np.float32)

    # per-group gather-call layout for the t gather (A chunk | B chunk,
    # rounded to GCH columns, trailing pad columns idx=-1 + num_idxs_reg)
    gmeta = []
    for (g0, g1) in groups:
        nA = sum(SA_list[g0:g1])
        nB = sum(SB_list[g0:g1])
        padA = math.ceil(nA / GCH) * GCH if nA else 0
        padB = math.ceil(nB / GCH) * GCH if nB else 0
        gmeta.append({"nA": nA, "nB": nB, "padA": padA, "padB": padB})
    wrap_off = np.concatenate(
        [[0], np.cumsum([m["padA"] + m["padB"] for m in gmeta])]).astype(int)
    febase = np.concatenate(
        [[0], np.cumsum([1 + (offs[g1] - offs[g0]) * TP
                         for (g0, g1) in groups])]).astype(int)
    FE_rows = int(febase[-1])

    in_maps = []
    for c in range(ncores):
        vids = np.arange(c * NPC, (c + 1) * NPC)
        wparts = []
        featE = np.zeros((FE_rows, D), np.float16)
        for gi, (g0, g1) in enumerate(groups):
            m = gmeta[gi]
            colsA = np.full((TP, m["padA"]), -1, np.int32)
            colsB = np.full((TP, m["padB"]), -1, np.int32)
            ca = cb = 0
            for j in range(g0, g1):
                vj = vids[j * TP : (j + 1) * TP]
                sa, sb = SA_list[j], SB_list[j]
                et = ell_t[vj]
                rl = real[vj]
                colsA[:, ca : ca + sa] = np.where(rl[:, :sa], et[:, :sa], 0)
                colsB[:, cb : cb + sb] = np.where(
                    rl[:, sa : sa + sb], et[:, sa : sa + sb], 0)
                # featE rows for this tile's columns
                a0 = int(offs[j] - offs[g0])
                fs = ell_fsrc[vj, : sa + sb]  # [TP, S_j]
                rows = (febase[gi] + 1 + (a0 + np.arange(sa + sb))[None, :] * TP
                        + np.arange(TP)[:, None])
                rmask = fs >= 0
                featE[rows[rmask]] = feat[fs[rmask]]
                ca += sa
                cb += sb
            wparts.append(_wrap_idx(np.concatenate([colsA, colsB], axis=1)))
        srctW = np.concatenate(wparts, axis=1) if wparts else np.zeros(
            (TP, 8), np.int16)

        # bigp in per-tile-contiguous (nd) layout
        bigp = np.full((TP, F_tot), NEG_BIG, np.float32)
        for j in range(NSLOT):
            vj = vids[j * TP : (j + 1) * TP]
            S_j = S_list[j]
            bigp[:, offs[j] : offs[j] + S_j] = np.where(
                real[vj, :S_j], 0.0, NEG_BIG)

        kfc = kf[vids].reshape(NSLOT, TP).T.astype(np.float32)
        degc = degf[vids].reshape(NSLOT, TP).T
        am = np.where(degc > 0, 1.0 / np.maximum(kfc, 1.0), 0.0).astype(np.float32)
        c1 = np.where(degc > 0, 1.0, 2.0).astype(np.float32)
        cmaskW = np.concatenate(
            [(np.arange(8 * R_list[j])[None, :] < kfc[:, j][:, None])
             for j in range(NSLOT)], axis=1).astype(np.float32)
        feat_pad = np.zeros((NPC, D), np.float32)
        vmask = valid[vids]
        feat_pad[vmask] = feat[fperm[vids[vmask]]]
        featT = feat_pad.T.copy()
        featN = (feat_pad.reshape(NSLOT, TP, D).transpose(1, 0, 2)
                 .reshape(TP, -1)).copy()
        m8 = (np.arange(TP)[:, None] // 16 == np.arange(8)[None, :]).astype(
            np.float32)
        m16 = (np.arange(TP)[:, None] % 16 == np.arange(TP)[None, :] % 16).astype(
            np.float32)
        in_maps.append(
            {
                "m8": m8,
                "m16": m16,
                "featT": featT,
                "featN": featN,
                "featE": featE,
                "srctW": srctW,
                "bigp": bigp,
                "kf": kfc,
                "am": am,
                "c1": c1,
                "cmaskW": cmaskW,
            }
        )

    sched = {
        "NTOT": NTOT,
        "NPC": NPC,
        "NSLOT": NSLOT,
        "SA": SA_list,
        "SB": SB_list,
        "S": S_list,
        "R": R_list,
        "offs": offs.tolist(),
        "F_tot": F_tot,
        "groups": groups,
        "gmeta": gmeta,
        "wrap_off": wrap_off.tolist(),
        "febase": febase.tolist(),
        "FE_rows": FE_rows,
        "SW": SW,
        "CW": CW,
        "c8off": np.concatenate(
            [[0], np.cumsum([8 * r for r in R_list])]).astype(int).tolist(),
    }
    return sched, in_maps, fperm, valid


# ----------------------------------------------------------------------------
# Bass program builder (SPMD: one program; per-core variation is data only)
# ----------------------------------------------------------------------------
def _build_bass(sched, ncores):
    STAGE = int(os.environ.get("K_STAGE", "99"))
    import concourse.bass as bass
    import concourse.bacc as bacc
    import concourse.tile as tile
    from concourse import mybir
    from concourse.masks import make_identity

    f32 = mybir.dt.float32
    f16 = mybir.dt.float16
    i16 = mybir.dt.int16
    CPH = 2 * CP  # fp16 t row: 128 halves = 256B
    NTOT, NPC, NSLOT = sched["NTOT"], sched["NPC"], sched["NSLOT"]
    SA_list, SB_list, S_list, R_list = (
        sched["SA"], sched["SB"], sched["S"], sched["R"],
    )
    offs, F_tot, groups = sched["offs"], sched["F_tot"], sched["groups"]
    gmeta, wrap_off, febase = sched["gmeta"], sched["wrap_off"], sched["febase"]
    FE_rows, SW, CW = sched["FE_rows"], sched["SW"], sched["CW"]
    c8off = sched["c8off"]
    R8_tot = c8off[-1]

    nc = bacc.Bacc(None, num_devices=ncores)

    featT = nc.dram_tensor("featT", [D, NPC], f32, kind="ExternalInput")
    featN = nc.dram_tensor("featN", [TP, NSLOT * D], f32, kind="ExternalInput")
    featE = nc.dram_tensor("featE", [FE_rows, D], f16, kind="ExternalInput")
    srctW = nc.dram_tensor("srctW", [TP, 8 * wrap_off[-1]], i16,
                           kind="ExternalInput")
    bigp = nc.dram_tensor("bigp", [TP, F_tot], f32, kind="ExternalInput")
    kf_d = nc.dram_tensor("kf", [TP, NSLOT], f32, kind="ExternalInput")
    cmask_d = nc.dram_tensor("cmaskW", [TP, R8_tot], f32, kind="ExternalInput")
    am_d = nc.dram_tensor("am", [TP, NSLOT], f32, kind="ExternalInput")
    c1_d = nc.dram_tensor("c1", [TP, NSLOT], f32, kind="ExternalInput")
    wmlp = nc.dram_tensor("wmlp", [D, C], f32, kind="ExternalInput")
    bmlp = nc.dram_tensor("bmlp", [C, 1], f32, kind="ExternalInput")
    wlin = nc.dram_tensor("wlin", [D, D], f32, kind="ExternalInput")
    blin = nc.dram_tensor("blin", [D, 1], f32, kind="ExternalInput")
    m8_d = nc.dram_tensor("m8", [TP, 8], f32, kind="ExternalInput")
    m16_d = nc.dram_tensor("m16", [TP, TP], f32, kind="ExternalInput")
    outT = nc.dram_tensor("outT", [D, NPC], f32, kind="ExternalOutput")

    with tile.TileContext(nc) as tc:
        with (
            tc.tile_pool(name="persist", bufs=1) as pp,
            tc.tile_pool(name="dram", bufs=1, space="DRAM") as dp,
        ):
            t_loc = dp.tile([NPC, CPH], f16)
            t_aug = dp.tile([NTOT, CPH], f16)

            ident = pp.tile([TP, TP], f32)
            make_identity(nc, ident[:])
            wmlp_t = pp.tile([D, C], f32)
            nc.sync.dma_start(wmlp_t[:], wmlp[:])
            bmlp_t = pp.tile([C, 1], f32)
            nc.sync.dma_start(bmlp_t[:], bmlp[:])
            wlin_t = pp.tile([D, D], f32)
            nc.sync.dma_start(wlin_t[:], wlin[:])
            blin_t = pp.tile([D, 1], f32)
            nc.sync.dma_start(blin_t[:], blin[:])
            kf_t = pp.tile([TP, NSLOT], f32)
            nc.sync.dma_start(kf_t[:], kf_d[:])
            cmask_t = pp.tile([TP, R8_tot], f32)
            nc.sync.dma_start(cmask_t[:], cmask_d[:])
            am_t = pp.tile([TP, NSLOT], f32)
            nc.sync.dma_start(am_t[:], am_d[:])
            c1_t = pp.tile([TP, NSLOT], f32)
            nc.sync.dma_start(c1_t[:], c1_d[:])
            m8_t = pp.tile([TP, 8], f32)
            nc.sync.dma_start(m8_t[:], m8_d[:])
            m16_t = pp.tile([TP, TP], f32)
            nc.sync.dma_start(m16_t[:], m16_d[:])

            iota_i = pp.tile([TP, SW], mybir.dt.int32)
            nc.gpsimd.iota(out=iota_i[:], pattern=[[1, SW]], base=0,
                           channel_multiplier=0)
            iota_f = pp.tile([TP, SW], f32)
            nc.vector.tensor_copy(out=iota_f[:], in_=iota_i[:])
            iotap_i = pp.tile([TP, 1], mybir.dt.int32)
            nc.gpsimd.iota(out=iotap_i[:], pattern=[[0, 1]], base=0,
                           channel_multiplier=1)
            iota_p = pp.tile([TP, 1], f32)
            nc.vector.tensor_copy(out=iota_p[:], in_=iotap_i[:])
            t_own = pp.tile([TP, NSLOT * C], f16)

            # ---------------- Phase A: t = tanh(feat @ W_mlp + b) ----------
            with (
                tc.tile_pool(name="pa", bufs=3) as pa,
                tc.tile_pool(name="psA", bufs=2, space="PSUM") as psA,
            ):
                o = 0
                while o < NPC:
                    w = min(512, NPC - o)
                    ft = pa.tile([D, w], f32, tag="ft")
                    nc.sync.dma_start(ft[:], featT[:, o : o + w])
                    pm = psA.tile([C, w], f32, tag="pm")
                    nc.tensor.matmul(out=pm[:], lhsT=wmlp_t[:], rhs=ft[:],
                                     start=True, stop=True)
                    th = pa.tile([C, w], f32, tag="th")
                    nc.scalar.activation(out=th[:], in_=pm[:],
                                         func=mybir.ActivationFunctionType.Tanh,
                                         bias=bmlp_t[:])
                    for i in range(w // TP):
                        j = (o + i * TP) // TP
                        pt = psA.tile([TP, C], f32, tag="pt")
                        nc.tensor.transpose(out=pt[:], in_=th[:, i * TP : (i + 1) * TP],
                                            identity=ident[:C, :C])
                        nc.vector.tensor_copy(out=t_own[:, j * C : (j + 1) * C],
                                              in_=pt[:])
                        nc.sync.dma_start(
                            t_loc[o + i * TP : o + (i + 1) * TP, :C],
                            t_own[:, j * C : (j + 1) * C],
                        )
                    o += w

            nc.gpsimd.collective_compute(
                "AllGather",
                mybir.AluOpType.bypass,
                replica_groups=[list(range(ncores))],
                ins=[t_loc[:].opt()],
                outs=[t_aug[:].opt()],
            )

            if STAGE < 4:
                for j in range(NSLOT):
                    nc.sync.dma_start(outT[:, j * TP : (j + 1) * TP], ident[:])
            # ---------------- Phase B ---------------------------------------
            # Software-pipelined: loads + t-gathers of group g+1 are emitted
            # before the compute of group g, so the in-order Pool queue can
            # generate the next group's gather descriptors while the DVE
            # works on the current group (the feat gather of group g is the
            # only Pool instruction that waits on DVE results).
            with (
                tc.tile_pool(name="pb", bufs=2) as pb,
                tc.tile_pool(name="pwmi", bufs=3) as pwmi,
                tc.tile_pool(name="pfin", bufs=4) as pfin,
                tc.tile_pool(name="pmsg", bufs=2) as pmsg,
                tc.tile_pool(name="psB", bufs=2, space="PSUM") as psB,
            ):

                def emit_load(gi):
                    j0, j1 = groups[gi]
                    nt = j1 - j0
                    Wg = offs[j1] - offs[j0]
                    gm = gmeta[gi]
                    nA, nB = gm["nA"], gm["nB"]
                    padA, padB = gm["padA"], gm["padB"]

                    big = pb.tile([TP, Wg], f32, tag="big")
                    nc.sync.dma_start(big[:], bigp[:, offs[j0] : offs[j1]])
                    fno = pfin.tile([TP, nt * D], f32, tag="fno")
                    nc.sync.dma_start(fno[:], featN[:, j0 * D : j1 * D])
                    wiT = pb.tile([TP, 8 * (padA + padB)], i16, tag="wiT")
                    nc.sync.dma_start(
                        wiT[:],
                        srctW[:, 8 * wrap_off[gi] : 8 * wrap_off[gi + 1]])

                    tsrc = pb.tile([TP, (padA + padB) * CPH], f16, tag="tsrc")
                    tsrc3 = tsrc[:].rearrange("p (w c) -> p w c", c=CPH)
                    for c0 in range(0, padA, GCH):
                        rcols = min(GCH, nA - c0)
                        nc.gpsimd.dma_gather(
                            out_ap=tsrc3[:, c0 : c0 + GCH, :],
                            in_ap=t_aug[:WIN, :],
                            idxs_ap=wiT[:, 8 * c0 : 8 * (c0 + GCH)],
                            num_idxs=GCH * TP, num_idxs_reg=rcols * TP,
                            elem_size=CPH,
                            single_packet=(GCH * TP <= 1024),
                        )
                    for c0 in range(0, padB, GCH):
                        rcols = min(GCH, nB - c0)
                        nc.gpsimd.dma_gather(
                            out_ap=tsrc3[:, padA + c0 : padA + c0 + GCH, :],
                            in_ap=t_aug[NTOT - WIN : NTOT, :],
                            idxs_ap=wiT[:, 8 * (padA + c0) : 8 * (padA + c0 + GCH)],
                            num_idxs=GCH * TP, num_idxs_reg=rcols * TP,
                            elem_size=CPH,
                            single_packet=(GCH * TP <= 1024),
                        )
                    return {"big": big, "fno": fno, "tsrc": tsrc,
                            "tsrc3": tsrc3}

                def emit_sel(gi, tl):
                    j0, j1 = groups[gi]
                    nt = j1 - j0
                    Wg = offs[j1] - offs[j0]
                    gm = gmeta[gi]
                    padA, padB = gm["padA"], gm["padB"]
                    R8g = sum(8 * R_list[j] for j in range(j0, j1))
                    big = tl["big"]
                    tsrc, tsrc3 = tl["tsrc"], tl["tsrc3"]

                    # subtract own-node t (broadcast over slots) per tile/part
                    for tci in range(nt):
                        j = j0 + tci
                        sa, sb = SA_list[j], SB_list[j]
                        tdst_b = t_own[:, j * C : (j + 1) * C]
                        cA0 = sum(SA_list[j0:j])
                        cB0 = padA + sum(SB_list[j0:j])
                        for (c0, ns) in ((cA0, sa), (cB0, sb)):
                            if ns == 0:
                                continue
                            sl = tsrc[:, c0 * CPH : (c0 + ns) * CPH].rearrange(
                                "p (s c) -> p s c", s=ns)[:, :, :C]
                            nc.vector.tensor_tensor(
                                out=sl, in0=sl,
                                in1=tdst_b.unsqueeze(1).to_broadcast([TP, ns, C]),
                                op=mybir.AluOpType.subtract,
                            )
                    ndg = pb.tile([TP, padA + padB], f32, tag="ndg")
                    nA_g, nB_g = gm["nA"], gm["nB"]
                    nc.vector.tensor_reduce(
                        out=ndg[:, :nA_g],
                        in_=tsrc3[:, :nA_g, :C],
                        axis=mybir.AxisListType.X, op=mybir.AluOpType.add,
                        apply_absolute_value=True, negate=True,
                    )
                    nc.vector.tensor_reduce(
                        out=ndg[:, padA : padA + nB_g],
                        in_=tsrc3[:, padA : padA + nB_g, :C],
                        axis=mybir.AxisListType.X, op=mybir.AluOpType.add,
                        apply_absolute_value=True, negate=True,
                    )
                    # per-tile contiguous negated dists (A|B adjacent) + pad
                    # mask (-1e30); top-R*8 per node via max8/match_replace;
                    # max_index recovers the slot positions (ties only for
                    # duplicate edges, where either index gives the same row).
                    ndw = pb.tile([TP, Wg], f32, tag="ndw")
                    for tci in range(nt):
                        j = j0 + tci
                        sa, sb = SA_list[j], SB_list[j]
                        a0 = offs[j] - offs[j0]
                        cA0 = sum(SA_list[j0:j])
                        cB0 = padA + sum(SB_list[j0:j])
                        if sa:
                            nc.vector.tensor_copy(out=ndw[:, a0 : a0 + sa],
                                                  in_=ndg[:, cA0 : cA0 + sa])
                        if sb:
                            nc.vector.tensor_copy(out=ndw[:, a0 + sa : a0 + sa + sb],
                                                  in_=ndg[:, cB0 : cB0 + sb])
                    nc.vector.tensor_tensor(out=ndw[:], in0=ndw[:], in1=big[:],
                                            op=mybir.AluOpType.add)
                    cand = pb.tile([TP, CW], f32, tag="cand")
                    idxu = pb.tile([TP, CW], mybir.dt.uint16, tag="idxu")
                    gsel = pb.tile([TP, R8g], f32, tag="gsel")
                    r8off = 0
                    for tci in range(nt):
                        j = j0 + tci
                        S_j = S_list[j]
                        R8j = 8 * R_list[j]
                        a0 = offs[j] - offs[j0]
                        ndw_j = ndw[:, a0 : a0 + S_j]
                        for r in range(R_list[j]):
                            nc.vector.max(out=cand[:, r * 8 : (r + 1) * 8],
                                          in_=ndw_j)
                            nc.vector.max_index(
                                out=idxu[:, r * 8 : (r + 1) * 8],
                                in_max=cand[:, r * 8 : (r + 1) * 8],
                                in_values=ndw_j)
                            if r < R_list[j] - 1:
                                nc.vector.match_replace(
                                    out=ndw_j,
                                    in_to_replace=cand[:, r * 8 : (r + 1) * 8],
                                    in_values=ndw_j, imm_value=NEG_BIG)
                        gsl = gsel[:, r8off : r8off + R8j]
                        # gidx = (a0 + slot)*128 + p + 1, zeroed for c >= k
                        nc.vector.tensor_copy(out=gsl, in_=idxu[:, :R8j])
                        nc.vector.tensor_scalar(
                            out=gsl, in0=gsl, scalar1=float(TP),
                            scalar2=float(a0 * TP + 1), op0=mybir.AluOpType.mult,
                            op1=mybir.AluOpType.add)
                        nc.vector.tensor_tensor(
                            out=gsl, in0=gsl,
                            in1=iota_p[:].to_broadcast([TP, R8j]),
                            op=mybir.AluOpType.add)
                        nc.vector.tensor_tensor(
                            out=gsl, in0=gsl,
                            in1=cmask_t[:, c8off[j] : c8off[j] + R8j],
                            op=mybir.AluOpType.mult)
                        r8off += R8j

                    # wrapped int16 transform of gsel for the whole group
                    bsel = pb.tile([TP, 8 * R8g], f32, tag="bsel")
                    nc.vector.tensor_tensor(
                        out=bsel[:].rearrange("p (s r) -> p s r", r=8),
                        in0=gsel[:].unsqueeze(2).to_broadcast([TP, R8g, 8]),
                        in1=m8_t[:].unsqueeze(1).to_broadcast([TP, R8g, 8]),
                        op=mybir.AluOpType.mult)
                    wmi = pwmi.tile([TP, 8 * R8g], i16, tag="wmi")
                    for ch in range(0, 8 * R8g, 512):
                        cw = min(512, 8 * R8g - ch)
                        pw = psB.tile([TP, 512], f32, tag="pwrap")
                        nc.tensor.matmul(out=pw[:, :cw], lhsT=m16_t[:],
                                         rhs=bsel[:, ch : ch + cw],
                                         start=True, stop=True)
                        nc.vector.tensor_copy(out=wmi[:, ch : ch + cw],
                                              in_=pw[:, :cw])
                    return {"wmi": wmi}

                def emit_fin(gi, tl, sl):
                    j0, j1 = groups[gi]
                    nt = j1 - j0
                    R8g = sum(8 * R_list[j] for j in range(j0, j1))
                    fno = tl["fno"]
                    wmi = sl["wmi"]

                    # feat gather per tile pair; per tile tree-add + linear
                    feg = featE[febase[gi] : febase[gi + 1], :]
                    r8off = 0
                    for tp0 in range(0, nt, 2):
                        tps = [t for t in (tp0, tp0 + 1) if t < nt]
                        R8p = sum(8 * R_list[j0 + t] for t in tps)
                        msgp = pmsg.tile([TP, R8p * D], f16, tag="msg")
                        nc.gpsimd.dma_gather(
                            out_ap=msgp[:].rearrange("p (s d) -> p s d", d=D),
                            in_ap=feg,
                            idxs_ap=wmi[:, 8 * r8off : 8 * (r8off + R8p)],
                            num_idxs=R8p * TP, num_idxs_reg=R8p * TP,
                            elem_size=D,
                            single_packet=(R8p * TP <= 1024),
                        )
                        r8off += R8p
                        moff = 0
                        for tci in tps:
                            emit_tile_fin(gi, tci, msgp, moff, fno)
                            moff += 8 * R_list[j0 + tci] * D

                def emit_tile_fin(gi, tci, msgp, moff, fno):
                    j0, j1 = groups[gi]
                    j = j0 + tci
                    R8j = 8 * R_list[j]
                    msg = msgp[:, moff : moff + R8j * D]
                    # contiguous tree-add over the R8j candidate columns
                    if True:
                        w = R8j
                        while w > 1:
                            h = w // 2
                            nc.vector.tensor_tensor(
                                out=msg[:, : h * D], in0=msg[:, : h * D],
                                in1=msg[:, h * D : 2 * h * D],
                                op=mybir.AluOpType.add)
                            if w % 2:
                                nc.vector.tensor_tensor(
                                    out=msg[:, :D], in0=msg[:, :D],
                                    in1=msg[:, (w - 1) * D : w * D],
                                    op=mybir.AluOpType.add)
                            w = h
                        h_t = pmsg.tile([TP, D], f32, tag="h")
                        nc.vector.tensor_scalar(
                            out=h_t[:], in0=msg[:, :D], scalar1=am_t[:, j : j + 1],
                            scalar2=None, op0=mybir.AluOpType.mult)
                        fown = pmsg.tile([TP, D], f32, tag="fown")
                        nc.vector.tensor_scalar(
                            out=fown[:], in0=fno[:, tci * D : (tci + 1) * D],
                            scalar1=c1_t[:, j : j + 1],
                            scalar2=None, op0=mybir.AluOpType.mult)
                        nc.vector.tensor_tensor(out=h_t[:], in0=h_t[:], in1=fown[:],
                                                op=mybir.AluOpType.add)
                        hT_p = psB.tile([TP, D], f32, tag="hT_p")
                        nc.tensor.transpose(out=hT_p[:], in_=h_t[:],
                                            identity=ident[:])
                        hT = pmsg.tile([TP, D], f32, tag="hT")
                        nc.vector.tensor_copy(out=hT[:], in_=hT_p[:])
                        op = psB.tile([D, TP], f32, tag="op")
                        nc.tensor.matmul(out=op[:], lhsT=wlin_t[:], rhs=hT[:],
                                         start=True, stop=True)
                        ot = pmsg.tile([D, TP], f32, tag="ot")
                        nc.scalar.activation(
                            out=ot[:], in_=op[:],
                            func=mybir.ActivationFunctionType.Identity,
                            bias=blin_t[:])
                        nc.sync.dma_start(outT[:, j * TP : (j + 1) * TP], ot[:])

                if STAGE > 1:
                    ng = len(groups)
                    LAG = 3  # fin runs LAG groups behind load
                    if PIPE:
                        tls, sls = {}, {}
                        for gi in range(ng + LAG):
                            if gi < ng:
                                tls[gi] = emit_load(gi)
                            if 1 <= gi <= ng and STAGE > 2:
                                sls[gi - 1] = emit_sel(gi - 1, tls[gi - 1])
                            if gi >= LAG and STAGE > 3:
                                emit_fin(gi - LAG, tls[gi - LAG], sls[gi - LAG])
                    else:
                        for gi in range(ng):
                            tl = emit_load(gi)
                            if STAGE > 2:
                                sl = emit_sel(gi, tl)
                                if STAGE > 3:
                                    emit_fin(gi, tl, sl)
    nc.finalize()
    return nc


# ----------------------------------------------------------------------------
# Runner
# ----------------------------------------------------------------------------
_CACHE = {}


def _get_program(sched, ncores):
    key = (tuple(sched["S"]), tuple(sched["SA"]), tuple(sched["R"]),
           sched["NTOT"], GCH, PIPE)
    if key not in _CACHE:
        _CACHE[key] = _build_bass(sched, ncores)
    return _CACHE[key]


def kernel(**inputs):
    feat = np.asarray(inputs["feat"], dtype=np.float32)
    src = np.asarray(inputs["src"]).astype(np.int64)
    dst = np.asarray(inputs["dst"]).astype(np.int64)
    W_mlp = np.asarray(inputs["W_mlp"], dtype=np.float32)
    b_mlp = np.asarray(inputs["b_mlp"], dtype=np.float32)
    W_lin = np.asarray(inputs["W_lin"], dtype=np.float32)
    b_lin = np.asarray(inputs["b_lin"], dtype=np.float32)
    n = feat.shape[0]

    sched, in_maps, fperm, valid = _preprocess(src, dst, feat, n, NCORES)
    for m in in_maps:
        m["wmlp"] = np.ascontiguousarray(W_mlp)
        m["bmlp"] = np.ascontiguousarray(b_mlp.reshape(C, 1))
        m["wlin"] = np.ascontiguousarray(W_lin)
        m["blin"] = np.ascontiguousarray(b_lin.reshape(D, 1))

    nc = _get_program(sched, NCORES)

    from concourse.bass_utils import run_bass_kernel_spmd

    res = run_bass_kernel_spmd(nc, in_maps, list(range(NCORES)))

    full = np.concatenate([res.results[c]["outT"] for c in range(NCORES)], axis=1)
    out = np.empty((n, D), np.float32)
    out[fperm[valid]] = full.T[valid]
    return out
